# revision 11
# baseline (speedup 1.0000x reference)
"""QRNN fo-pooling kernel for Trainium2 (Bass/Tile), batch-sharded across 8 cores.

Reference computation (per (b, h) element, sequential over t):
    F, Z, O = split(Y, 3, axis=2); F = sigmoid(F); Z = tanh(Z); O = sigmoid(O)
    c_t = F_t * c_{t-1} + (1 - F_t) * Z_t
    h_t = O_t * c_t
    out = concat([init_h, h], axis=0)

v2 design (slab loads + batched engines), per batch b and t-chunk j:
  - one HWDGE load pulls the F+Z halves of Y[j*128:(j+1)*128, b, :] as a
    [128, 2048] slab: 8KB contiguous per partition (vs 512B runs), 3x fewer
    HWDGE dispatches than per-gate loads
  - PE transposes 128x128 chunks into ps_f/ps_z PSUM tiles [128, 1024]
    (hb-major), ACT drains them with N=1024 activations (sigmoid(-F), tanh)
  - Pool computes f = 1 - s_neg, DVE computes zf = s_neg * zt, both writing
    full-b tensors [128, HB, TJ, 128] so the recurrence runs as ONE
    tensor_tensor_scan of N=512 per (b, hb) (free dims (j, t) iterate in
    exact time order)
  - phase 3 per (b, j): PE transposes c back to natural layout in [128,512]
    chunks, O is loaded late (4KB runs), ACT sigmoids it, DVE multiplies,
    and the store goes out through SWDGE (gpsimd) with 2KB contiguous rows,
    keeping both HWDGE rings free for loads
"""

import numpy as np

import concourse.bacc as bacc
import concourse.bass as bass
import concourse.mybir as mybir
import concourse.tile as tile
from concourse.bass_utils import run_bass_kernel_spmd
from concourse.masks import make_identity


T, B, H = 512, 32, 1024
N_CORES = 8
BS = B // N_CORES  # batches per core
P = 128
HB = H // P  # h-blocks
TJ = T // P  # t-chunks

FP32 = mybir.dt.float32

_nc_cache = []


def _build_bass(repeat: int = 1, dma_only: bool = False) -> bass.Bass:
    nc = bacc.Bacc("TRN2", target_bir_lowering=False)
    y = nc.declare_dram_parameter("Y", [T, BS, 3 * H], FP32, isOutput=False)
    init_c = nc.declare_dram_parameter("init_c", [1, BS, H], FP32, isOutput=False)
    init_h = nc.declare_dram_parameter("init_h", [1, BS, H], FP32, isOutput=False)
    out = nc.declare_dram_parameter("out", [T + 1, BS, H], FP32, isOutput=True)

    with tile.TileContext(nc) as tc:
        with (
            tc.tile_pool(name="sb", bufs=3) as sb,
            tc.tile_pool(name="psum", bufs=2, space="PSUM") as psum,
            tc.tile_pool(name="singles", bufs=1) as singles,
        ):
            ident = singles.tile([P, P], FP32)
            make_identity(nc, ident)

            # out[0] = init_h[0] (row 0 of the output is the initial h)
            nc.sync.dma_start(out=out[0, :, :], in_=init_h[0, :, :])

            # [t, b, c] -> [p, j, b, c] with t = j*128 + p
            yr = y[:, :, :].rearrange("(j p) b c -> p j b c", p=P)
            outr = out[1 : T + 1, :, :].rearrange("(j p) b h -> p j b h", p=P)
            # all initial states in one load: [p=h%128, b, hb]
            ic_all = singles.tile([P, BS, HB], FP32)
            nc.sync.dma_start(
                out=ic_all,
                in_=init_c[0, :, :].rearrange("b (hb p) -> p b hb", p=P),
            )

            def _dma_only_body():
                # measurement probe: identical DMA traffic, no compute
                zero = singles.tile([P, H], FP32, tag="zero")
                nc.vector.memset(zero, 0.0)
                for b in range(BS):
                    for j in range(TJ):
                        fz = sb.tile([P, 2 * H], FP32, tag="fz", bufs=3)
                        nc.sync.dma_start(out=fz, in_=yr[:, j, b, 0 : 2 * H])
                        osl = sb.tile([P, H], FP32, tag="osl", bufs=3)
                        nc.sync.dma_start(out=osl, in_=yr[:, j, b, 2 * H : 3 * H])
                        for half in range(2):
                            nc.gpsimd.dma_start(
                                out=outr[
                                    :, j, b,
                                    half * (H // 2) : (half + 1) * (H // 2),
                                ],
                                in_=zero[:, half * (H // 2) : (half + 1) * (H // 2)],
                            )

            def _rep_body():
                if dma_only:
                    _dma_only_body()
                    return
                for b in range(BS):
                    # c keeps full-b layout [p=h%128, hb, j, t%128] (phase 3
                    # and the chunk-chain initial read it across j)
                    c_t = sb.tile([P, HB, TJ, P], FP32, tag="c_t", bufs=2)

                    for j in range(TJ):
                        # phase 1: load F/Z slabs (4KB contiguous rows),
                        # prefetch O, transpose, activations. Deep load
                        # buffering so the DMA queue never stalls on tiles.
                        fsl = sb.tile([P, H], FP32, tag="fsl", bufs=6)
                        nc.sync.dma_start(out=fsl, in_=yr[:, j, b, 0:H])
                        zsl = sb.tile([P, H], FP32, tag="zsl", bufs=6)
                        nc.sync.dma_start(out=zsl, in_=yr[:, j, b, H : 2 * H])
                        osl = sb.tile([P, H], FP32, tag="osl", bufs=6)
                        nc.sync.dma_start(out=osl, in_=yr[:, j, b, 2 * H : 3 * H])

                        ps_f = psum.tile([P, H], FP32, tag="ps_f", bufs=2)
                        ps_z = psum.tile([P, H], FP32, tag="ps_z", bufs=1)
                        for hb in range(HB):
                            nc.tensor.transpose(
                                ps_f[:, hb * P : (hb + 1) * P],
                                fsl[:, hb * P : (hb + 1) * P],
                                ident,
                            )
                        for hb in range(HB):
                            nc.tensor.transpose(
                                ps_z[:, hb * P : (hb + 1) * P],
                                zsl[:, hb * P : (hb + 1) * P],
                                ident,
                            )

                        # ACT drains PSUM: s_neg = 1 - sigmoid(F), zt = tanh(Z)
                        s_neg = sb.tile([P, H], FP32, tag="s_neg", bufs=3)
                        nc.scalar.activation(
                            s_neg, ps_f, mybir.ActivationFunctionType.Sigmoid,
                            scale=-1.0,
                        )
                        zt = sb.tile([P, H], FP32, tag="zt", bufs=3)
                        nc.scalar.activation(
                            zt, ps_z, mybir.ActivationFunctionType.Tanh
                        )
                        # sigmoid(O) early so phase 3 never waits on ACT
                        o_sig = sb.tile([P, H], FP32, tag="o_sig", bufs=3)
                        nc.scalar.activation(
                            o_sig, osl, mybir.ActivationFunctionType.Sigmoid
                        )

                        sr = s_neg.rearrange("p (hb t) -> p hb t", hb=HB)
                        zr = zt.rearrange("p (hb t) -> p hb t", hb=HB)
                        # per-j gate tensors [p, hb, t] — consumed by this j's
                        # scans immediately, no full-b persistence needed
                        f_t = sb.tile([P, HB, P], FP32, tag="f_t", bufs=3)
                        zf = sb.tile([P, HB, P], FP32, tag="zf", bufs=3)
                        # f = 1 - s_neg on Pool; zf = s_neg * tanh(z) on DVE
                        nc.gpsimd.tensor_scalar(
                            f_t, sr, -1.0, 1.0,
                            op0=mybir.AluOpType.mult, op1=mybir.AluOpType.add,
                        )
                        nc.vector.tensor_mul(zf, zr, sr)

                        # phase 2: chained chunk scans — c for this t-chunk is
                        # ready as soon as this chunk's gates are, instead of
                        # waiting for the whole sequence
                        for hb in range(HB):
                            nc.vector.tensor_tensor_scan(
                                c_t[:, hb, j, :],
                                f_t[:, hb, :],
                                zf[:, hb, :],
                                initial=(
                                    ic_all[:, b, hb : hb + 1]
                                    if j == 0
                                    else c_t[:, hb, j - 1, P - 1 : P]
                                ),
                                op0=mybir.AluOpType.mult,
                                op1=mybir.AluOpType.add,
                            )

                        # phase 3: c back to natural layout, h = sigmoid(O)*c
                        for half in range(2):
                            ps_c = psum.tile([P, H // 2], FP32, tag="ps_c", bufs=2)
                            for hh in range(HB // 2):
                                hb = half * (HB // 2) + hh
                                nc.tensor.transpose(
                                    ps_c[:, hh * P : (hh + 1) * P],
                                    c_t[:, hb, j, :],
                                    ident,
                                )
                            h_out = sb.tile([P, H // 2], FP32, tag="h_out", bufs=4)
                            nc.vector.tensor_mul(
                                h_out,
                                o_sig[:, half * (H // 2) : (half + 1) * (H // 2)],
                                ps_c,
                            )
                            # store via SWDGE: 2KB contiguous rows, keeps the
                            # HWDGE rings free for loads
                            nc.gpsimd.dma_start(
                                out=outr[
                                    :, j, b,
                                    half * (H // 2) : (half + 1) * (H // 2),
                                ],
                                in_=h_out,
                            )

            if repeat == 1:
                _rep_body()
            else:
                # timing mode: hardware loop keeps the NEFF size constant in
                # `repeat`, so two loop bounds can be wall-clock diffed
                with tc.For_i(0, repeat, 1):
                    _rep_body()
    nc.compile()
    return nc


def _get_nc() -> bass.Bass:
    if not _nc_cache:
        _nc_cache.append(_build_bass())
    return _nc_cache[0]


def kernel(Y: np.ndarray, init_c: np.ndarray, init_h: np.ndarray) -> np.ndarray:
    Y = np.ascontiguousarray(np.asarray(Y, dtype=np.float32))
    init_c = np.ascontiguousarray(np.asarray(init_c, dtype=np.float32))
    init_h = np.ascontiguousarray(np.asarray(init_h, dtype=np.float32))

    in_maps = []
    for k in range(N_CORES):
        sl = slice(k * BS, (k + 1) * BS)
        in_maps.append(
            {
                "Y": np.ascontiguousarray(Y[:, sl, :]),
                "init_c": np.ascontiguousarray(init_c[:, sl, :]),
                "init_h": np.ascontiguousarray(init_h[:, sl, :]),
            }
        )

    nc = _get_nc()
    res = run_bass_kernel_spmd(nc, in_maps, core_ids=list(range(N_CORES)))
    return np.concatenate([r["out"] for r in res.results], axis=1)
